# revision 1
# baseline (speedup 1.0000x reference)
"""Bass/Trainium2 kernel for nn_ContrastiveLoss_18502719111626.

Reference math:
    mask_i = (sum_d latent[i,d] != 0)
    ln     = latent / max(||latent_i||, 1e-8)
    total  = einsum('i,ij,j->', mask, ln @ ln.T, mask) - sum(mask)
    out    = 0.01 * total / (2 * N)

Key identity: einsum('i,ij,j->', m, ln@ln.T, m) == ||sum_i m_i * ln_i||^2,
so the N x N similarity matrix is never needed. Each core streams its
1024-row shard once (memory-roofline), producing a 64-dim weighted
column sum s_c and a mask count c_c. Host combines:
    total = ||sum_c s_c||^2 - sum_c c_c.

Per-core dataflow (shard [1024, 64] f32):
    X[128, 512] sbuf, col-group g = shard rows g*128..g*128+127 (8 DMAs)
    ss8[p,g] = sum_d X[p, g*64+d]^2    (8 ScalarE Square ops w/ accum_out)
    rs8[p,g] = sum_d X[p, g*64+d]      (1 VectorE reduce over [128,8,64])
    scale8 = (rs8 != 0) / max(sqrt(ss8), eps)
    psum_s[1,64] += scale8[:,g].T @ X[:,g*64:(g+1)*64]   (8 accumulating matmuls)
    psum_c[1,1]  = cnt_per_partition.T @ ones            (1 matmul)
    partials[1,65] = [s | cnt] -> DRAM
"""

import numpy as np

N = 8192
D = 64
NCORES = 8
ROWS = N // NCORES  # 1024 rows per core
GROUPS = ROWS // 128  # 8 column-groups of the sbuf tile
COF1 = 0.01
EPS = 1e-8

_prog = None


def _build(n_in_dmas=2):
    import concourse.bacc as bacc
    import concourse.mybir as mybir
    import concourse.tile as tile

    f32 = mybir.dt.float32
    AF = mybir.ActivationFunctionType
    ALU = mybir.AluOpType

    # Bacc (not plain Bass): its compile() runs generate_event_semaphores,
    # which splits multi-sem sync waits into EventSemaphore instructions --
    # walrus rejects >1 wait per instruction.
    nc = bacc.Bacc(None)
    x_in = nc.declare_dram_parameter("latent", [ROWS, D], f32, isOutput=False)
    out_p = nc.declare_dram_parameter("partials", [1, D + 1], f32, isOutput=True)

    with tile.TileContext(nc) as tc:
        with (
            tc.tile_pool(name="sbuf", bufs=1) as pool,
            tc.tile_pool(name="psum", bufs=1, space="PSUM") as psum_pool,
        ):
            X = pool.tile([128, GROUPS * D], f32)
            # Column-group g holds shard rows g*128..g*128+127 (256B
            # contiguous per partition). Few dma_starts: the kernel-tail
            # drain and the result-store DMA have limited sync-wait slots,
            # so total DMA-queue usage must stay small.
            gs = GROUPS // n_in_dmas  # groups per dma_start
            for c in range(n_in_dmas):
                nc.sync.dma_start(
                    out=X[:, c * gs * D : (c + 1) * gs * D].rearrange(
                        "p (g d) -> p g d", g=gs
                    ),
                    in_=x_in[c * gs * 128 : (c + 1) * gs * 128, :].rearrange(
                        "(g p) d -> p g d", p=128
                    ),
                )

            ones = pool.tile([128, 1], f32)
            nc.vector.memset(ones[:], 1.0)

            # Dummy sqrt as ScalarE's first instruction: pulls in the
            # "sqrt_and_others" activation table (which also contains
            # square), so only one ACT_TABLE_LOAD happens, early, instead
            # of a second 1.3us load mid-kernel right before the real sqrt.
            warm = pool.tile([128, 1], f32)
            nc.scalar.sqrt(warm[:], ones[:])

            # Row sum-of-squares per group on ScalarE (overlaps the
            # serialized DMA triggers; VectorE handles the row sums).
            sq = pool.tile([128, GROUPS * D], f32)
            ss8 = pool.tile([128, GROUPS], f32)
            for g in range(GROUPS):
                nc.scalar.activation(
                    out=sq[:, g * D : (g + 1) * D],
                    in_=X[:, g * D : (g + 1) * D],
                    func=AF.Square,
                    accum_out=ss8[:, g : g + 1],
                )

            # Row sums per group on VectorE. The copy output also launders
            # the DMA deps away from the PE (matmuls read xcopy).
            xcopy = pool.tile([128, GROUPS * D], f32)
            rs8 = pool.tile([128, GROUPS], f32)
            for g in range(GROUPS):
                nc.vector.tensor_scalar(
                    xcopy[:, g * D : (g + 1) * D],
                    X[:, g * D : (g + 1) * D],
                    1.0, 0.0,
                    op0=ALU.mult, op1=ALU.add,
                    accum_out=rs8[:, g : g + 1],
                )

            # scale = (rs != 0) / max(sqrt(ss), eps); cnt via accum of mask.
            # max(sqrt(ss), eps) == sqrt(max(ss, eps^2)) since ss >= 0.
            ssc = pool.tile([128, GROUPS], f32)
            nc.vector.tensor_scalar_max(ssc[:], ss8[:], EPS * EPS)
            norm = pool.tile([128, GROUPS], f32)
            nc.scalar.sqrt(norm[:], ssc[:])
            mask = pool.tile([128, GROUPS], f32)
            cntp = pool.tile([128, 1], f32)
            nc.vector.tensor_scalar(
                mask[:], rs8[:], 0.0, 0.0,
                op0=ALU.not_equal, op1=ALU.add, accum_out=cntp[:],
            )
            inv = pool.tile([128, GROUPS], f32)
            nc.vector.reciprocal(inv[:], norm[:])
            scale8 = pool.tile([128, GROUPS], f32)
            nc.vector.tensor_mul(scale8[:], inv[:], mask[:])

            # s[1,64]: weighted column sums, accumulated in PSUM over groups.
            psum_s = psum_pool.tile([1, D], f32)
            for g in range(GROUPS):
                nc.tensor.matmul(
                    psum_s[:],
                    scale8[:, g : g + 1],
                    xcopy[:, g * D : (g + 1) * D],
                    start=(g == 0),
                    stop=(g == GROUPS - 1),
                )
            psum_c = psum_pool.tile([1, 1], f32)
            nc.tensor.matmul(psum_c[:], cntp[:], ones[:], start=True, stop=True)

            res = pool.tile([1, D + 1], f32)
            nc.vector.tensor_copy(res[:, :D], psum_s[:])
            nc.vector.tensor_copy(res[:, D : D + 1], psum_c[:])
            nc.sync.dma_start(out=out_p[:, :], in_=res[:])

    nc.compile()
    return nc


def _run_spmd(latent, trace=False, **kw):
    from concourse.bass_utils import run_bass_kernel_spmd

    global _prog
    if _prog is None:
        _prog = _build()
    in_maps = [
        {"latent": np.ascontiguousarray(latent[c * ROWS : (c + 1) * ROWS])}
        for c in range(NCORES)
    ]
    return run_bass_kernel_spmd(_prog, in_maps, list(range(NCORES)), trace=trace, **kw)


def _combine(results):
    parts = np.stack([results[c]["partials"][0] for c in range(NCORES)])  # [8, 65]
    s = parts[:, :D].astype(np.float64).sum(axis=0)
    cnt = parts[:, D].astype(np.float64).sum()
    total = float(s @ s - cnt)
    return np.asarray(COF1 * total / (2.0 * N), dtype=np.float32)


def kernel(latent):
    latent = np.asarray(latent, dtype=np.float32)
    assert latent.shape == (N, D)
    return _combine(_run_spmd(latent).results)



# revision 2
# speedup vs baseline: 1.1121x; 1.1121x over previous
"""Bass/Trainium2 kernel for nn_ContrastiveLoss_18502719111626.

Reference math:
    mask_i = (sum_d latent[i,d] != 0)
    ln     = latent / max(||latent_i||, 1e-8)
    total  = einsum('i,ij,j->', mask, ln @ ln.T, mask) - sum(mask)
    out    = 0.01 * total / (2 * N)

Key identity: einsum('i,ij,j->', m, ln@ln.T, m) == ||sum_i m_i * ln_i||^2,
so the N x N similarity matrix is never needed. Each core streams its
1024-row shard once (memory-roofline) and emits per-partition partials
res[128, 65] = [sum_g scale[p,g]*X[p,g,:] | count[p]]; the host combines
    s = sum_{c,p} res[c,p,:64];  cnt = sum res[...,64]
    total = ||s||^2 - cnt.

Per-core dataflow (shard [1024, 64] f32, partition p holds rows 8p..8p+7):
    X[128, 8*64] sbuf  (ONE dma, 128 x 2KB contiguous descriptors)
    sq  = X^2                        ScalarE, one whole-tile Square
    rs8[p,g] = sum_d X[p,g,d]        VectorE tensor_reduce axis=X
    ss8[p,g] = sum_d sq[p,g,d]       VectorE tensor_reduce axis=X
    scale8 = (rs8 != 0) / sqrt(max(ss8, eps^2))   (sqrt on ScalarE,
             reciprocal_approx_fast on VectorE; accum gives count)
    prod = X * scale8(bcast over d)  VectorE
    res[:, :64] = sum_g prod[p,g,:]  VectorE tensor_reduce (permuted view)
    res[128, 65] -> DRAM

Engines used: Sync(DMA), Scalar, Vector. No PE/PSUM, no GpSimd --
minimizes cross-engine events (the kernel-end event cleanup is
proportional to total event count).
"""

import numpy as np

N = 8192
D = 64
NCORES = 8
ROWS = N // NCORES  # 1024 rows per core
GROUPS = ROWS // 128  # 8 row-groups per partition
COF1 = 0.01
EPS = 1e-8

_prog = None


def _build():
    import concourse.bacc as bacc
    import concourse.mybir as mybir
    import concourse.tile as tile

    f32 = mybir.dt.float32
    ALU = mybir.AluOpType
    AX = mybir.AxisListType

    # Bacc (not plain Bass): its compile() runs generate_event_semaphores,
    # which splits multi-sem sync waits into EventSemaphore instructions --
    # walrus rejects >1 wait per instruction.
    nc = bacc.Bacc(None)
    x_in = nc.declare_dram_parameter("latent", [ROWS, D], f32, isOutput=False)
    out_p = nc.declare_dram_parameter("partials", [128, D + 1], f32, isOutput=True)

    with tile.TileContext(nc) as tc:
        with tc.tile_pool(name="sbuf", bufs=1) as pool:
            # Dummy sqrt as ScalarE's first instruction pulls the sqrt
            # activation table in early (overlapping the input DMA) so no
            # 1.3us ACT_TABLE_LOAD lands mid-kernel before the real sqrt.
            warm = pool.tile([1, 1], f32)
            nc.vector.memset(warm[:], 1.0)
            warm2 = pool.tile([1, 1], f32)
            nc.scalar.sqrt(warm2[:], warm[:])

            # Partition p holds shard rows 8p..8p+7: per-partition source is
            # 2KB contiguous DRAM -> one descriptor per partition.
            X = pool.tile([128, GROUPS * D], f32)
            nc.sync.dma_start(
                out=X[:, :],
                in_=x_in.rearrange("(p f) d -> p (f d)", p=128),
            )

            Xg = X[:, :].rearrange("p (g d) -> p g d", g=GROUPS)

            # Row sums per group (for the mask) on VectorE; squares on
            # ScalarE in parallel.
            rs8 = pool.tile([128, GROUPS], f32)
            nc.vector.reduce_sum(rs8[:, :], Xg, axis=AX.X)
            sq = pool.tile([128, GROUPS * D], f32)
            nc.scalar.square(sq[:, :], X[:, :])
            ss8 = pool.tile([128, GROUPS], f32)
            nc.vector.reduce_sum(
                ss8[:, :], sq[:, :].rearrange("p (g d) -> p g d", g=GROUPS), axis=AX.X
            )

            # scale = (rs != 0) / max(sqrt(ss), eps); count via accum of mask.
            # max(sqrt(ss), eps) == sqrt(max(ss, eps^2)) since ss >= 0.
            ssc = pool.tile([128, GROUPS], f32)
            nc.vector.tensor_scalar_max(ssc[:, :], ss8[:, :], EPS * EPS)
            nrm = pool.tile([128, GROUPS], f32)
            nc.scalar.sqrt(nrm[:, :], ssc[:, :])

            res = pool.tile([128, D + 1], f32)
            mask = pool.tile([128, GROUPS], f32)
            nc.vector.tensor_scalar(
                mask[:, :], rs8[:, :], 0.0, 0.0,
                op0=ALU.not_equal, op1=ALU.add, accum_out=res[:, D : D + 1],
            )
            # norms are ~sqrt(64); approx reciprocal (~51 ULP) is far inside
            # the error budget and 5x faster than nc.vector.reciprocal.
            inv = pool.tile([128, GROUPS], f32)
            nc.vector.reciprocal_approx_fast(inv[:, :], nrm[:, :])
            sc8 = pool.tile([128, GROUPS], f32)
            nc.vector.tensor_mul(sc8[:, :], inv[:, :], mask[:, :])

            # prod[p,g,d] = scale[p,g] * X[p,g,d]; then reduce over g with a
            # permuted view so res[p,d] = sum_g prod[p,g,d].
            prod = pool.tile([128, GROUPS * D], f32)
            scb = sc8[:, :].rearrange("p (g o) -> p g o", o=1).broadcast_to(
                [128, GROUPS, D]
            )
            nc.vector.tensor_mul(prod[:, :].rearrange("p (g d) -> p g d", g=GROUPS),
                                 Xg, scb)
            nc.vector.reduce_sum(
                res[:, :D],
                prod[:, :].rearrange("p (g d) -> p d g", g=GROUPS),
                axis=AX.X,
            )

            nc.sync.dma_start(out=out_p[:, :], in_=res[:, :])

    nc.compile()
    return nc


def _run_spmd(latent, trace=False, **kw):
    from concourse.bass_utils import run_bass_kernel_spmd

    global _prog
    if _prog is None:
        _prog = _build()
    in_maps = [
        {"latent": np.ascontiguousarray(latent[c * ROWS : (c + 1) * ROWS])}
        for c in range(NCORES)
    ]
    return run_bass_kernel_spmd(_prog, in_maps, list(range(NCORES)), trace=trace, **kw)


def _combine(results):
    parts = np.stack([results[c]["partials"] for c in range(NCORES)])  # [8, 128, 65]
    s = parts[:, :, :D].astype(np.float64).sum(axis=(0, 1))
    cnt = parts[:, :, D].astype(np.float64).sum()
    total = float(s @ s - cnt)
    return np.asarray(COF1 * total / (2.0 * N), dtype=np.float32)


def kernel(latent):
    latent = np.asarray(latent, dtype=np.float32)
    assert latent.shape == (N, D)
    return _combine(_run_spmd(latent).results)


# revision 6
# speedup vs baseline: 1.1413x; 1.0262x over previous
"""Bass/Trainium2 kernel for nn_ContrastiveLoss_18502719111626.

Reference math:
    mask_i = (sum_d latent[i,d] != 0)
    ln     = latent / max(||latent_i||, 1e-8)
    total  = einsum('i,ij,j->', mask, ln @ ln.T, mask) - sum(mask)
    out    = 0.01 * total / (2 * N)

Key identity: einsum('i,ij,j->', m, ln@ln.T, m) == ||sum_i m_i * ln_i||^2,
so the N x N similarity matrix is never needed. Each core streams its
1024-row shard once (memory-roofline) and emits per-partition partials
res[128, 65] = [sum_g scale[p,g]*X[p,g,:] | count[p]]; the host combines
    s = sum_{c,p} res[c,p,:64];  cnt = sum res[...,64]
    total = ||s||^2 - cnt.

Per-core dataflow (shard [1024, 64] f32, partition p holds rows 8p..8p+7):
    X[128, 8*64] sbuf, loaded via TWO parallel HWDGE rings (SP ring gets
    partitions 0..63, Activation ring 64..127; 2KB contiguous descriptors)
    sq = X^2                       ScalarE, two half-tile Squares
    rs8[p,g] = sum_d X[p,g,d]      VectorE tensor_reduce axis=X (2 halves)
    ss8[p,g] = sum_d sq[p,g,d]     VectorE tensor_reduce axis=X (2 halves)
    nrm = sqrt(ss8 + eps^2)        ScalarE (bias trick; == max form since
                                   ss>=0 and eps^2 vanishes in fp32)
    inv = ~1/nrm                   VectorE reciprocal_approx_fast (51 ULP)
    sc8 = (rs8 != 0) * inv         VectorE scalar_tensor_tensor, one op
    cnt  (GpSimd, off critical path) accum of (rs8 != 0)
    prod = X * sc8(bcast d)        groups 0-3 on VectorE, 4-7 on GpSimd
    res[:, :64] = add-tree over g  VectorE, 3 contiguous tensor_adds
    res[128, 65] -> DRAM

The kernel-end walrus event-teardown (~7.7us) and DMA trigger->data
latencies are fixed costs of this execution path; the structure above
minimizes everything in between.
"""

import numpy as np

N = 8192
D = 64
NCORES = 8
ROWS = N // NCORES  # 1024 rows per core
GROUPS = ROWS // 128  # 8 row-groups per partition
HG = GROUPS // 2
COF1 = 0.01
EPS = 1e-8

_prog = None


def _build():
    import concourse.bacc as bacc
    import concourse.mybir as mybir
    import concourse.tile as tile

    f32 = mybir.dt.float32
    AF = mybir.ActivationFunctionType
    ALU = mybir.AluOpType
    AX = mybir.AxisListType

    # Bacc (not plain Bass): its compile() runs generate_event_semaphores,
    # which splits multi-sem sync waits into EventSemaphore instructions --
    # walrus rejects >1 wait per instruction.
    nc = bacc.Bacc(None)
    x_in = nc.declare_dram_parameter("latent", [ROWS, D], f32, isOutput=False)
    out_p = nc.declare_dram_parameter("partials", [128, D + 1], f32, isOutput=True)

    with tile.TileContext(nc) as tc:
        with tc.tile_pool(name="sbuf", bufs=1) as pool:
            # Partition p holds shard rows 8p..8p+7 (2KB contiguous per
            # partition). Two dma_starts on different HWDGE rings (SP and
            # Activation) so the descriptor streams run in parallel.
            X = pool.tile([128, GROUPS * D], f32)
            xv = x_in.rearrange("(p f) d -> p (f d)", p=128)
            nc.sync.dma_start(out=X[0:64, :], in_=xv[0:64, :])
            nc.scalar.dma_start(out=X[64:128, :], in_=xv[64:128, :])

            # Dummy sqrt as ScalarE's next instruction pulls the sqrt
            # activation table in early (overlapping the input DMA) so no
            # 1.3us ACT_TABLE_LOAD lands mid-kernel before the real sqrt.
            warm = pool.tile([1, 1], f32)
            nc.vector.memset(warm[:], 1.0)
            warm2 = pool.tile([1, 1], f32)
            nc.scalar.sqrt(warm2[:], warm[:])
            # Per-partition eps^2 bias for the fused sqrt(ss + eps^2).
            epsb = pool.tile([128, 1], f32)
            nc.vector.memset(epsb[:], EPS * EPS)

            Xg = X[:, :].rearrange("p (g d) -> p g d", g=GROUPS)

            # Row sums per group (for the mask) on VectorE.
            rs8 = pool.tile([128, GROUPS], f32)
            nc.vector.reduce_sum(rs8[:, :HG], Xg[:, :HG], axis=AX.X)
            nc.vector.reduce_sum(rs8[:, HG:], Xg[:, HG:], axis=AX.X)

            # Squares on ScalarE (overlaps the Vector reduces), halves so
            # the first reduce can start before the second square is done.
            sq = pool.tile([128, GROUPS * D], f32)
            half = HG * D
            nc.scalar.square(sq[:, :half], X[:, :half])
            nc.scalar.square(sq[:, half:], X[:, half:])
            sqg = sq[:, :].rearrange("p (g d) -> p g d", g=GROUPS)
            ss8 = pool.tile([128, GROUPS], f32)
            nc.vector.reduce_sum(ss8[:, :HG], sqg[:, :HG], axis=AX.X)
            nc.vector.reduce_sum(ss8[:, HG:], sqg[:, HG:], axis=AX.X)

            # max(sqrt(ss), eps) == sqrt(ss + eps^2) here: for ss >> eps^2
            # the +eps^2 vanishes in fp32; for ss ~ 0 it clamps to eps.
            nrm = pool.tile([128, GROUPS], f32)
            nc.scalar.activation(nrm[:, :], ss8[:, :], AF.Sqrt, bias=epsb[:, :])

            # norms are ~sqrt(64); approx reciprocal (~51 ULP) is far inside
            # the error budget and 5x faster than nc.vector.reciprocal.
            inv = pool.tile([128, GROUPS], f32)
            nc.vector.reciprocal_approx_fast(inv[:, :], nrm[:, :])
            # scale = (rs != 0) * inv in a single fused op.
            sc8 = pool.tile([128, GROUPS], f32)
            nc.vector.scalar_tensor_tensor(
                sc8[:, :], rs8[:, :], 0.0, inv[:, :],
                op0=ALU.not_equal, op1=ALU.mult,
            )

            # Count of participating rows (accum of the row mask). Runs on
            # VectorE early, overlapped with the ScalarE squares.
            res = pool.tile([128, D + 1], f32)
            mjunk = pool.tile([128, GROUPS], f32)
            nc.vector.tensor_scalar(
                mjunk[:, :], rs8[:, :], 0.0, 0.0,
                op0=ALU.not_equal, op1=ALU.add, accum_out=res[:, D : D + 1],
            )

            # prod[p,g,d] = sc8[p,g] * X[p,g,d]; VectorE takes groups 0-3,
            # GpSimd 4-7 concurrently.
            prod = pool.tile([128, GROUPS * D], f32)
            pg = prod[:, :].rearrange("p (g d) -> p g d", g=GROUPS)
            scb = sc8[:, :].rearrange("p (g o) -> p g o", o=1).broadcast_to(
                [128, GROUPS, D]
            )
            nc.vector.tensor_mul(pg[:, :HG], Xg[:, :HG], scb[:, :HG])
            nc.gpsimd.tensor_mul(pg[:, HG:], Xg[:, HG:], scb[:, HG:])

            # res[p,d] = sum_g prod[p,g,d] as a contiguous add-tree (faster
            # than one strided tensor_reduce over the permuted view).
            t1 = pool.tile([128, 4 * D], f32)
            nc.vector.tensor_add(t1[:, :], prod[:, : 4 * D], prod[:, 4 * D :])
            t2 = pool.tile([128, 2 * D], f32)
            nc.vector.tensor_add(t2[:, :], t1[:, : 2 * D], t1[:, 2 * D :])
            nc.vector.tensor_add(res[:, :D], t2[:, :D], t2[:, D:])

            nc.sync.dma_start(out=out_p[:, :], in_=res[:, :])

    nc.compile()
    return nc


def _run_spmd(latent, trace=False, **kw):
    from concourse.bass_utils import run_bass_kernel_spmd

    global _prog
    if _prog is None:
        _prog = _build()
    in_maps = [
        {"latent": np.ascontiguousarray(latent[c * ROWS : (c + 1) * ROWS])}
        for c in range(NCORES)
    ]
    return run_bass_kernel_spmd(_prog, in_maps, list(range(NCORES)), trace=trace, **kw)


def _combine(results):
    parts = np.stack([results[c]["partials"] for c in range(NCORES)])  # [8, 128, 65]
    s = parts[:, :, :D].astype(np.float64).sum(axis=(0, 1))
    cnt = parts[:, :, D].astype(np.float64).sum()
    total = float(s @ s - cnt)
    return np.asarray(COF1 * total / (2.0 * N), dtype=np.float32)


def kernel(latent):
    latent = np.asarray(latent, dtype=np.float32)
    assert latent.shape == (N, D)
    return _combine(_run_spmd(latent).results)
